# revision 42
# baseline (speedup 1.0000x reference)
"""Trainium2 Bass kernel for a bidirectional selective-scan SSM (Mamba-like).

Problem: nn_ProMU_42623255445559
  B=8, L=2048, D=256, N=16, R=16
  Data-parallel over batch: core i handles batch row i; weights replicated.

v3 dataflow (d on partitions, l in free; two 128-partition halves):
  x_dbl^T = Wxp @ x^T                  (PE)
  delta   = softplus(Wdt @ delta_r^T + b_dt) = ln(exp(z)+1)   (ACT exp+ln,
            single act-func table: ln/exp/copy/identity share set 6)
  delta_b computed in FORWARD order from x (not xf); consumers read it with
            reversed APs, so xf^T is never materialized.
  a_n     = exp(A_n * delta)           (ACT, per-partition scale = A_n < 0)
  b_n     = u*Bf_n + ub_rev*Bb_n       (DVE bf16 2x; u=delta*x, ub=delta_b*x)
  h_n     = scan(a, b) along l         (Pool engine; DVE stays on mults)
  yg      = tree-reduce_n (h_n * C_n)  (DVE bf16 2x, per n-group of 8)
  out     = (yg0 + yg1 + (x+xf)*D_skip) @ W_out^T
            -- assembled in PSUM: 6 accumulating bf16 matmuls (PE)

Host-side prep: weight transposes, A=-exp(A_log), +b_dt, bf16 W_out.
"""

import sys

sys.path.insert(0, "/opt/trn_rl_repo")

from contextlib import ExitStack

import numpy as np

import concourse.bacc as bacc
import concourse.bass as bass
import concourse.mybir as mybir
import concourse.tile as tile
from concourse import bass_utils
from concourse.bass import AP

B, L, D, N, R = 8, 2048, 256, 16, 16
PROJ = R + 3 * N  # 64 rows of x_dbl^T
FP32 = mybir.dt.float32
BF16 = mybir.dt.bfloat16
AF = mybir.ActivationFunctionType
ALU = mybir.AluOpType

NCORES = 8
LC = 512          # l-chunk for the scan pipeline
NLC = L // LC     # 4
NG = 8            # n per group
G = N // NG       # 2 groups
LSUB = 128        # l-subchunk for out-proj matmuls

# which (c, g, h) iterations run their reduce tree on Pool (balance tuning)
TREE_POOL = {(c, g, h) for c in range(NLC) for g in range(G) for h in range(2)}
# scans are DVE-only (TPB ISA rejects the scan opcode on Pool)
SCAN_POOL = set()


def _rev_ap(ap2d):
    """Reverse the (single) free dim of a [P, F] AP."""
    (pstep, pcount), (fstep, fcount) = ap2d.ap
    assert fstep == 1
    return AP(ap2d.tensor, ap2d.offset + fcount - 1, [[pstep, pcount], [-1, fcount]])


def _rep_ap(ap2d, r):
    """Repeat a [P, F] AP r times along free -> [P, r, F] with stride 0."""
    (pstep, pcount), (fstep, fcount) = ap2d.ap
    assert fstep == 1
    return AP(ap2d.tensor, ap2d.offset, [[pstep, pcount], [0, r], [1, fcount]])


def _rep_rev_ap(ap2d, r):
    """Repeat the REVERSED [P, F] AP r times along free -> [P, r, F]."""
    (pstep, pcount), (fstep, fcount) = ap2d.ap
    assert fstep == 1
    return AP(ap2d.tensor, ap2d.offset + fcount - 1,
              [[pstep, pcount], [0, r], [-1, fcount]])


def _blk_ap(ap2d, r, f):
    """View a [P, r*f] AP as [P, r, f]."""
    (pstep, pcount), (fstep, fcount) = ap2d.ap
    assert fstep == 1 and fcount == r * f
    return AP(ap2d.tensor, ap2d.offset, [[pstep, pcount], [f, r], [1, f]])


def _emit(tc, nc, io):
    x_d, wxpT_d, wxbT_d, wdtT_d, bdt_d, aneg_d, dskip_d, woutT_d, eye_d, out_d = io

    ctx = ExitStack()
    with ctx:
        const = ctx.enter_context(tc.tile_pool(name="const", bufs=1))
        big = ctx.enter_context(tc.tile_pool(name="big", bufs=1))
        tps = ctx.enter_context(tc.tile_pool(name="tps", bufs=2, space="PSUM"))
        mmp = ctx.enter_context(tc.tile_pool(name="mmp", bufs=2, space="PSUM"))
        ops = ctx.enter_context(tc.tile_pool(name="ops", bufs=2, space="PSUM"))
        ldp = ctx.enter_context(tc.tile_pool(name="ldp", bufs=3))
        wk = ctx.enter_context(tc.tile_pool(name="wk", bufs=2))
        drp = ctx.enter_context(tc.tile_pool(name="drp", bufs=1, space="DRAM"))

        # ---- constants (all pre-transposed host-side) ------------------
        eye = const.tile([128, 128], FP32, tag="eye")
        nc.sync.dma_start(eye[:, :], eye_d[:, :])
        # x loads issued before the other consts (they gate the prologue)
        xldp = []
        for cq in range(4):
            xn = ldp.tile([128, 4 * D], FP32, tag="ld4", bufs=2)
            s = x_d[cq * 512:cq * 512 + 128, :]
            src4 = AP(s.tensor, s.offset,
                      [[s.ap[0][0], 128], [128 * s.ap[0][0], 4], [1, D]])
            dst4 = AP(xn.tensor, xn[:, :].offset,
                      [[xn[:, :].ap[0][0], 128], [D, 4], [1, D]])
            nc.sync.dma_start(dst4, src4)
            xldp.append(xn)

        wxpT = [const.tile([128, PROJ], BF16, name=f"wxpT{h}", tag=f"wxpT{h}")
                for h in range(2)]
        wxbT = [const.tile([128, R], BF16, name=f"wxbT{h}", tag=f"wxbT{h}")
                for h in range(2)]
        woutT = [const.tile([128, D], BF16, name=f"woutT{h}", tag=f"woutT{h}")
                 for h in range(2)]
        aneg = [const.tile([128, N], FP32, name=f"aneg{h}", tag=f"aneg{h}")
                for h in range(2)]
        bdt = [const.tile([128, 1], FP32, name=f"bdt{h}", tag=f"bdt{h}")
               for h in range(2)]
        dskip = [const.tile([128, 1], FP32, name=f"dsk{h}", tag=f"dsk{h}")
                 for h in range(2)]
        for h in range(2):
            hs = slice(h * 128, (h + 1) * 128)
            nc.sync.dma_start(wxpT[h][:, :], wxpT_d[hs, :])
            nc.sync.dma_start(wxbT[h][:, :], wxbT_d[hs, :])
            nc.sync.dma_start(woutT[h][:, :], woutT_d[hs, :])
            nc.sync.dma_start(aneg[h][:, :], aneg_d[hs, :])
            nc.sync.dma_start(bdt[h][:, :], bdt_d[hs, :])
            nc.sync.dma_start(dskip[h][:, :], dskip_d[hs, :])
        wdtT = const.tile([R, D], BF16, tag="wdtT")
        nc.sync.dma_start(wdtT[:, :], wdtT_d[:, :])

        # pre-touch DMA'd weights on PE so later matmuls don't accumulate
        # more sync-wait commands than the ISA allows
        warm = tps.tile([128, 128], FP32, tag="tps")
        nc.tensor.transpose(warm[:, :], eye[:, :], eye[:, :])
        warm2 = tps.tile([128, 128], FP32, tag="tps")
        nc.tensor.matmul(warm2[:, :], eye[:, :], eye[:, :],
                         start=True, stop=True)

        # ---- x^T ------------------------------------------------------
        xT = [big.tile([128, L], BF16, name=f"xT{h}", tag=f"xT{h}") for h in range(2)]
        for cq in range(4):
            xn = xldp[cq]
            for i4 in range(4):
                i = cq * 4 + i4
                for h in range(2):
                    pt = tps.tile([128, 128], FP32, tag="tps")
                    nc.tensor.transpose(pt[:, :],
                                        xn[:, i4 * D + h * 128:i4 * D + (h + 1) * 128],
                                        eye[:, :])
                    nc.vector.tensor_copy(
                        xT[h][:, i * 128:(i + 1) * 128], pt[:, :])

        # ---- projections + delta path (per LC chunk) -------------------
        # B/C rows of x_dbl (bf16) staged in DRAM; broadcasts read from there.
        # exp/ln phases are batched so the ACT engine never swaps func tables
        # (Exp lives in set 0, Ln in set 5, Copy in every set).
        xdbd = drp.tile([3 * N, L], BF16, tag="xdbd")
        zf = [big.tile([128, L], BF16, name=f"zf{h}", tag=f"zf{h}")
              for h in range(2)]
        zb = [big.tile([128, L], BF16, name=f"zb{h}", tag=f"zb{h}")
              for h in range(2)]
        dT = zf    # softplus closes in place: dT aliases zf, dbT aliases zb
        ubT = [big.tile([128, L], BF16, name=f"ubT{h}", tag=f"ubT{h}")
               for h in range(2)]
        xsk = [big.tile([128, L], BF16, name=f"xsk{h}", tag=f"xsk{h}")
               for h in range(2)]

        for c in range(NLC):
            sl = slice(c * LC, (c + 1) * LC)
            # x_dbl^T chunk (64, LC) = Wxp @ x^T
            pd = mmp.tile([128, LC], FP32, tag="mmp", bufs=3)
            for h in range(2):
                nc.tensor.matmul(pd[0:PROJ, :], wxpT[h][:, :], xT[h][:, sl],
                                 start=(h == 0), stop=(h == 1))
            # fp32 delta_r rows for the dt matmul; bf16 B/C rows -> DRAM
            drc = wk.tile([R, LC], BF16, tag="drc", bufs=1)
            nc.vector.tensor_copy(drc[:, :], pd[0:R, :])
            bcc = wk.tile([PROJ, LC], BF16, tag="bcc")
            nc.vector.tensor_copy(bcc[:, :], pd[0:PROJ, :])
            nc.sync.dma_start(xdbd[:, sl], bcc[R:PROJ, :])
            # xb^T chunk (16, LC) = W_xbproj @ x^T  (FORWARD order)
            pb = mmp.tile([128, LC], FP32, tag="mmp", bufs=3)
            for h in range(2):
                nc.tensor.matmul(pb[0:R, :], wxbT[h][:, :], xT[h][:, sl],
                                 start=(h == 0), stop=(h == 1))
            xbc = wk.tile([R, LC], BF16, tag="xbc", bufs=1)
            nc.vector.tensor_copy(xbc[:, :], pb[0:R, :])
            for h in range(2):
                hsl = slice(h * 128, (h + 1) * 128)
                # z = W_dt @ delta_r^T (+b_dt later); staged to SBUF by Pool
                pz = mmp.tile([128, LC], FP32, tag="mmp", bufs=3)
                nc.tensor.matmul(pz[:, :], wdtT[:, hsl], drc[:, :],
                                 start=True, stop=True)
                nc.scalar.activation(zf[h][:, sl], pz[:, :], AF.Exp,
                                     bias=bdt[h][:, 0:1])
                pz2 = mmp.tile([128, LC], FP32, tag="mmp", bufs=3)
                nc.tensor.matmul(pz2[:, :], wdtT[:, hsl], xbc[:, :],
                                 start=True, stop=True)
                nc.scalar.activation(zb[h][:, sl], pz2[:, :], AF.Exp,
                                     bias=bdt[h][:, 0:1])

        # delta = ln(exp(z + b_dt) + 1) [softplus]: full-L exp then ln per
        # direction-half -- 8 ACT instrs, no act-table swaps mid-stream
        dbT = zb
        for h in range(2):
            nc.scalar.activation(dT[h][:, :], zf[h][:, :], AF.Ln, bias=1.0)
            nc.scalar.activation(dbT[h][:, :], zb[h][:, :], AF.Ln, bias=1.0)
        for c in range(NLC):
            sl = slice(c * LC, (c + 1) * LC)
            rsl = slice(L - (c + 1) * LC, L - c * LC)
            for h in range(2):
                # ub = delta_b * x (forward order; read reversed later)
                nc.gpsimd.tensor_mul(ubT[h][:, sl], dbT[h][:, sl],
                                     xT[h][:, sl])
                # skip term (x + xf) * D_skip -> bf16 (matmul lhsT later)
                xs = wk.tile([128, LC], BF16, tag="ez")
                nc.gpsimd.tensor_add(xs[:, :], xT[h][:, sl],
                                     _rev_ap(xT[h][:, rsl]))
                nc.scalar.activation(xsk[h][:, sl], xs[:, :], AF.Copy,
                                     scale=dskip[h][:, 0:1])

        # ---- main scan loop ------------------------------------------
        def issue_reps(c, g):
            """Broadcast the (c, g) B/C n-rows to 128 partitions (prefetched
            one group ahead; rep tiles are double-buffered)."""
            sl_ = slice(c * LC, (c + 1) * LC)
            n0 = g * NG
            bf_rep = wk.tile([128, NG * LC], BF16, tag="bfr")
            bb_rep = wk.tile([128, NG * LC], BF16, tag="bbr")
            c_rep = wk.tile([128, NG * LC], BF16, tag="ccr")
            for rep, r0, qeng in ((bf_rep, n0, nc.sync),
                                  (bb_rep, N + n0, nc.sync),
                                  (c_rep, 2 * N + n0, nc.scalar)):
                s = xdbd[r0:r0 + NG, sl_]
                src_b = AP(s.tensor, s.offset,
                           [[0, 128], [s.ap[0][0], NG], [1, LC]])
                qeng.dma_start(_blk_ap(rep[:, :], NG, LC), src_b)
            return (bf_rep, bb_rep, c_rep)

        iters = [(c, g, h) for c in range(NLC) for g in range(G)
                 for h in range(2)]
        reps_of = {}
        carry = [[None, None], [None, None]]    # [g][h] -> carry cols tile
        u_cur = {}                              # (c, h) -> u chunk tile
        st = {}                                 # (c,g,h) -> stage-A tiles
        tree = {}                               # (c,g,h) -> y-part tile

        def ensure_reps(c, g):
            if (c, g) not in reps_of:
                reps_of[(c, g)] = issue_reps(c, g)
            return reps_of[(c, g)]

        def next_group(c, g):
            if g + 1 < G:
                return (c, g + 1)
            return (c + 1, 0) if c + 1 < NLC else None

        def stage_a(c, g, h):
            """a-cube exps (ACT), u mult, p/b products (DVE), badd (Pool)."""
            sl = slice(c * LC, (c + 1) * LC)
            rsl = slice(L - (c + 1) * LC, L - c * LC)
            n0 = g * NG
            bf_rep, bb_rep, c_rep = ensure_reps(c, g)
            if h == 0:
                ng = next_group(c, g)
                if ng:
                    ensure_reps(*ng)
            if (c, h) not in u_cur:
                ut = wk.tile([128, LC], BF16, tag=f"ut{h}", bufs=2)
                nc.vector.tensor_mul(ut[:, :], dT[h][:, sl], xT[h][:, sl])
                u_cur[(c, h)] = ut
            a_t = wk.tile([128, NG * LC], BF16, tag="at")
            for j in range(NG):
                n = n0 + j
                nc.scalar.activation(a_t[:, j * LC:(j + 1) * LC],
                                     dT[h][:, sl], AF.Exp,
                                     scale=aneg[h][:, n:n + 1])
            # ptm doubles as p-product scratch and later h*C tree buf
            ptm = wk.tile([128, NG * LC], BF16, tag="tm", bufs=3)
            b_t = wk.tile([128, NG * LC], BF16, tag="bt", bufs=3)
            nc.vector.tensor_tensor(_blk_ap(ptm[:, :], NG, LC),
                                    _rep_ap(u_cur[(c, h)][:, :], NG),
                                    _blk_ap(bf_rep[:, :], NG, LC), ALU.mult)
            nc.vector.tensor_tensor(_blk_ap(b_t[:, :], NG, LC),
                                    _rep_rev_ap(ubT[h][:, rsl], NG),
                                    _blk_ap(bb_rep[:, :], NG, LC), ALU.mult)
            st[(c, g, h)] = (a_t, b_t, ptm, c_rep)

        def stage_badd(c, g, h):
            # emitted with skew-1: its DVE inputs are complete, so it never
            # head-of-line-blocks the Pool queue
            a_t, b_t, ptm, c_rep = st[(c, g, h)]
            nc.gpsimd.tensor_add(b_t[:, :], b_t[:, :], ptm[:, :])

        def stage_b(c, g, h):
            """scans (DVE), carry snapshot + h*C tree reduce (Pool)."""
            a_t, b_t, ptm, c_rep = st.pop((c, g, h))
            h_t = wk.tile([128, NG * LC], BF16, tag="ht", bufs=2)
            for j in range(NG):
                js = slice(j * LC, (j + 1) * LC)
                if c == 0:
                    init = 0.0
                else:
                    init = carry[g][h][:, j:j + 1]
                nc.vector.tensor_tensor_scan(h_t[:, js], a_t[:, js],
                                             b_t[:, js], init,
                                             ALU.mult, ALU.add)
            if c < NLC - 1:
                cy = wk.tile([128, NG], BF16, tag=f"cy{g}{h}", bufs=2)
                nc.gpsimd.tensor_copy(
                    cy[:, :], AP(h_t.tensor, h_t[:, :].offset + LC - 1,
                                 [[h_t[:, :].ap[0][0], 128], [LC, NG]]))
                carry[g][h] = cy
            teng = (nc.vector if (c, g, h) == (NLC - 1, G - 1, 1)
                    else nc.gpsimd)
            tmp = ptm
            teng.tensor_mul(tmp[:, :], h_t[:, :], c_rep[:, :])
            half = NG * LC // 2
            while half >= 2 * LC:
                teng.tensor_add(tmp[:, 0:half], tmp[:, 0:half],
                                tmp[:, half:2 * half])
                half //= 2
            yg = wk.tile([128, LC], BF16, tag=f"yg{g}{h}", bufs=2)
            teng.tensor_add(yg[:, :], tmp[:, 0:LC], tmp[:, LC:2 * LC])
            tree[(c, g, h)] = yg
            if (g, h) == (G - 1, 1):
                out_proj(c)

        def out_proj(c):
            # psum accumulates (yg0 + yg1 + xsk) @ W_out^T per l-subchunk
            for s in range(LC // LSUB):
                l0 = c * LC + s * LSUB
                ssl = slice(s * LSUB, (s + 1) * LSUB)
                pt = ops.tile([LSUB, D], FP32, tag="ops")
                terms = []
                for h in range(2):
                    terms += [(xsk[h][:, l0:l0 + LSUB], h),
                              (tree[(c, 0, h)][:, ssl], h),
                              (tree[(c, 1, h)][:, ssl], h)]
                for k, (term, h) in enumerate(terms):
                    nc.tensor.matmul(pt[:, :], term, woutT[h][:, :],
                                     start=(k == 0), stop=(k == len(terms) - 1))
                ot = wk.tile([LSUB, D], FP32, tag="osb")
                nc.scalar.copy(ot[:, :], pt[:, :])
                nc.sync.dma_start(out_d[l0:l0 + LSUB, :], ot[:, :])

        # software-pipeline: products A(i+2), then badd(i+1) (skew-1, Pool),
        # then B(i) -- no engine head-of-line-blocks on a cross-engine handoff
        stage_a(*iters[0])
        stage_a(*iters[1])
        stage_badd(*iters[0])
        for k, it in enumerate(iters):
            if k + 2 < len(iters):
                stage_a(*iters[k + 2])
            if k + 1 < len(iters):
                stage_badd(*iters[k + 1])
            stage_b(*it)


_NC_CACHE = {}  # v3


def _build():
    if "nc" in _NC_CACHE:
        return _NC_CACHE["nc"]
    nc = bacc.Bacc("TRN2", target_bir_lowering=False, debug=False,
                   num_devices=NCORES)
    x_d = nc.dram_tensor("x", [L, D], FP32, kind="ExternalInput").ap()
    wxpT_d = nc.dram_tensor("WxpT", [D, PROJ], BF16, kind="ExternalInput").ap()
    wxbT_d = nc.dram_tensor("WxbT", [D, R], BF16, kind="ExternalInput").ap()
    wdtT_d = nc.dram_tensor("WdtT", [R, D], BF16, kind="ExternalInput").ap()
    bdt_d = nc.dram_tensor("bdt", [D, 1], FP32, kind="ExternalInput").ap()
    aneg_d = nc.dram_tensor("Aneg", [D, N], FP32, kind="ExternalInput").ap()
    dskip_d = nc.dram_tensor("Dskip", [D, 1], FP32, kind="ExternalInput").ap()
    woutT_d = nc.dram_tensor("WoutT", [D, D], BF16, kind="ExternalInput").ap()
    eye_d = nc.dram_tensor("eye", [128, 128], FP32, kind="ExternalInput").ap()
    out_d = nc.dram_tensor("out", [L, D], FP32, kind="ExternalOutput").ap()
    io = (x_d, wxpT_d, wxbT_d, wdtT_d, bdt_d, aneg_d, dskip_d, woutT_d,
          eye_d, out_d)
    with tile.TileContext(nc) as tc:
        _emit(tc, nc, io)
    nc.compile()
    _NC_CACHE["nc"] = nc
    return nc


def host_prep(W_xproj, W_xbproj, W_dt, b_dt, A_log, D_skip, W_out):
    """Host-side input transforms shared by all cores."""
    import ml_dtypes

    return {
        "WxpT": np.ascontiguousarray(
            np.asarray(W_xproj, dtype=np.float32).T.astype(ml_dtypes.bfloat16)),
        "WxbT": np.ascontiguousarray(
            np.asarray(W_xbproj, dtype=np.float32).T.astype(ml_dtypes.bfloat16)),
        "WdtT": np.ascontiguousarray(
            np.asarray(W_dt, dtype=np.float32).T.astype(ml_dtypes.bfloat16)),
        "bdt": np.ascontiguousarray(
            np.asarray(b_dt, dtype=np.float32).reshape(D, 1)),
        "Aneg": np.ascontiguousarray(
            -np.exp(np.asarray(A_log, dtype=np.float32))),
        "Dskip": np.ascontiguousarray(
            np.asarray(D_skip, dtype=np.float32).reshape(D, 1)),
        "WoutT": np.ascontiguousarray(
            np.asarray(W_out, dtype=np.float32).T.astype(ml_dtypes.bfloat16)),
        "eye": np.eye(128, dtype=np.float32),
    }


def kernel(x, W_xproj, W_xbproj, W_dt, b_dt, A_log, D_skip, W_out, **profile_kw):
    nc = _build()
    shared = host_prep(W_xproj, W_xbproj, W_dt, b_dt, A_log, D_skip, W_out)
    xs = np.asarray(x, dtype=np.float32)
    in_maps = [{"x": np.ascontiguousarray(xs[b]), **shared} for b in range(NCORES)]
    res = bass_utils.run_bass_kernel_spmd(nc, in_maps, core_ids=list(range(NCORES)),
                                          **profile_kw)
    out = np.stack([res.results[b]["out"] for b in range(NCORES)], axis=0)
    kernel.last_result = res
    return out
